# revision 19
# baseline (speedup 1.0000x reference)
"""COIL-style sparse-attention scoring kernel for Trainium2 (8 NeuronCores).

Reference computation:
    scores[q,i,d,j] = <query_tok_embs[q,i], doc_tok_embs[d,j]>         (K=32)
    masked = where(query_ids[q,i]==doc_ids[d,j], scores, 0)
    tok    = masked.max(axis=j)                                        (192 -> 1)
    tok_scores[q,d] = sum_i w[q,i] * tok[q,i,d]    (w drops CLS + SEP)
    out = tok_scores + query_cls_emb @ doc_cls_emb.T

Strategy (bucketed): only same-token-id pairs survive the mask, so
partition the vocabulary into NB=16 buckets (host-side greedy balance) and
compute scores ONLY within a bucket.  Sharding is over docs (16 per core);
queries are replicated so each bucket's <=128 active query tokens fill one
full 128-row PE block.  Per (core, bucket) the doc side has ~12 positions
per doc, padded to P columns -> a [49, 128] x [49, 16*P] fp16 matmul.

  * Match masking via digit one-hots: within bucket b, distinct active
    query-token ids get dense codes (<= 129 incl. a doc-only sentinel),
    4 base-4 digits -> 16 one-hot dims (query side scaled by C=128, doc
    side 1.0).  The matmul output is aug = score + 128 * (#match digits);
    a full 4-digit match carries +512 while partials stay < 448 + |score|
    (|score| < 60 for this data).
  * Decode WITHOUT an activation: contraction row 48 is 512 on the query
    side and hits 1.0 exactly once per (doc,bucket) segment (in the pad
    column every segment has), so every segment contains an aug == 512
    sentinel and VectorE reduce_max yields  512 + relu(best matched score)
    directly in fp16.  The spurious 512*n_q offset (n_q = active tokens of
    query q) is cancelled by a 7th CLS chunk with dcls=-4, qcls=n_q.
  * Weighted token sum + CLS: fp16 matmuls accumulating into a TRANSPOSED
    [16, 64] PSUM tile (so the output DMA moves 16 fat rows, not 64 thin
    ones).  Host re-transposes and concatenates per-core outputs.

Per core: 16 score matmuls x 224 cols, 4 VectorE segmented reduces, 23
accumulation matmuls, one [16,64] copy + DMA out.  Scalar and Pool issue
DMAs only; the Act table load disappears from the preamble.
"""

import os
import numpy as np
import ml_dtypes
from contextlib import ExitStack

import concourse.bass as bass
import concourse.bacc as bacc
import concourse.mybir as mybir
import concourse.tile as tile
from concourse.bass_utils import run_bass_kernel_spmd

F32 = mybir.dt.float32
F16 = mybir.dt.float16
BF16 = mybir.dt.bfloat16

# problem shape (hardcoded per contract)
BQ, LQ, BD, LD, TOK_D, CLS_D = 64, 32, 128, 192, 32, 768
VOCAB = 5000
NCORES = 8
DPC = BD // NCORES          # 16 docs per core
NB = 16                     # vocab buckets == PE blocks
NDIG, DIGB = 4, 4           # 4 base-4 digits -> 16 one-hot dims, 256 codes
KD = NDIG * DIGB            # 16
K = TOK_D + KD              # 48 data contraction dims
KK = K + 1                  # +1 sentinel row (512-seed)
C = 128.0                   # per-digit match bonus
OFF = NDIG * C              # 512 full-match offset
NST = 4                     # supertiles (4 buckets each) = 1 PSUM tile each
BPST = NB // NST            # blocks per supertile
REG = 256                   # psum cols per block region (half-bank aligned)
NCLS = 7                    # 6 real CLS chunks + 1 offset-correction chunk

USE_BF16 = os.environ.get("K_DTYPE", "fp16") == "bf16"
DT = BF16 if USE_BF16 else F16
NPDT = ml_dtypes.bfloat16 if USE_BF16 else np.float16


def build_nc(P):
    """P = padded positions per (doc, bucket); NCOL = DPC*P streamed cols."""
    NCOL = DPC * P
    assert NCOL <= REG
    nc = bacc.Bacc(
        "TRN2",
        target_bir_lowering=False,
        debug=False,
        num_devices=NCORES,
    )

    lhsT_d = nc.dram_tensor("lhsT", [KK, NB * 128], DT, kind="ExternalInput")
    rhs_d = nc.dram_tensor("rhs", [KK, NB * NCOL], DT, kind="ExternalInput")
    sel_d = nc.dram_tensor("sel", [128, NB * BQ], DT, kind="ExternalInput")
    qclsT_d = nc.dram_tensor("qclsT", [128, NCLS * BQ], DT, kind="ExternalInput")
    dclsT_d = nc.dram_tensor("dclsT", [128, NCLS * DPC], DT, kind="ExternalInput")
    out_d = nc.dram_tensor("out", [DPC, BQ], F32, kind="ExternalOutput")

    with tile.TileContext(nc) as tc, ExitStack() as ctx:
        const = ctx.enter_context(tc.tile_pool(name="const", bufs=1))
        psum = ctx.enter_context(tc.tile_pool(name="psum", bufs=3, space="PSUM"))
        opsum = ctx.enter_context(tc.tile_pool(name="opsum", bufs=1, space="PSUM"))
        work = ctx.enter_context(tc.tile_pool(name="work", bufs=1))

        lhsT_t = const.tile([KK, NB * 128], DT, tag="lhsT")
        rhs_t = const.tile([KK, NB * NCOL], DT, tag="rhs")
        sel_t = const.tile([128, NB * BQ], DT, tag="sel")
        qclsT_t = const.tile([128, NCLS * BQ], DT, tag="qclsT")
        dclsT_t = const.tile([128, NCLS * DPC], DT, tag="dclsT")

        # --- input DMAs.  All queues share the same 16 hw DMA engines;
        # throughput scales with per-row packet size, so use FEW FAT chunks
        # (plus tiny lead chunks so the PE starts immediately), ordered by
        # consumption: rhs/lhsT, then sel, then CLS.  gpsimd keeps only the
        # output DMA so it is uncontended at the end.
        nc.scalar.dma_start(rhs_t[:, 0 : 2 * NCOL], rhs_d[:, 0 : 2 * NCOL])
        nc.sync.dma_start(lhsT_t[:, 0 : 2 * 128], lhsT_d[:, 0 : 2 * 128])
        nc.scalar.dma_start(
            rhs_t[:, 2 * NCOL : 9 * NCOL], rhs_d[:, 2 * NCOL : 9 * NCOL]
        )
        nc.sync.dma_start(lhsT_t[:, 2 * 128 :], lhsT_d[:, 2 * 128 :])
        nc.scalar.dma_start(rhs_t[:, 9 * NCOL :], rhs_d[:, 9 * NCOL :])
        nc.sync.dma_start(sel_t[:], sel_d[:])
        nc.scalar.dma_start(dclsT_t[:], dclsT_d[:])
        nc.sync.dma_start(qclsT_t[:], qclsT_d[:])

        # --- emission: score supertiles pipelined with the DVE reduces and
        # the accumulation matmuls, ordered so the PE never head-of-line
        # blocks and the tail after the last score matmul is short.
        ps_tiles = [None] * NST
        tokdec = [None] * NST
        out_ps = opsum.tile([DPC, BQ], F32, tag="out_ps")

        def emit_score(st):
            ps = psum.tile([128, BPST, REG], F32, tag="score")
            ps_tiles[st] = ps
            for k in range(BPST):
                b = st * BPST + k
                nc.tensor.matmul(
                    ps[:, k, 0:NCOL],
                    lhsT_t[:, b * 128 : (b + 1) * 128],
                    rhs_t[:, b * NCOL : (b + 1) * NCOL],
                    start=True,
                    stop=True,
                )

        def emit_reduce(st):
            ps = ps_tiles[st]
            v = ps[:, :, 0:NCOL].rearrange("p k (d j) -> p k d j", j=P)
            dec = work.tile([128, BPST * DPC], DT, tag=f"dec{st}")
            nc.vector.reduce_max(
                dec[:].rearrange("p (k d) -> p k d", d=DPC),
                v,
                axis=mybir.AxisListType.X,
            )
            tokdec[st] = dec

        def emit_sel(st, first=False, last=False):
            # transposed accumulation: tokdec stationary, sel moving ->
            # out [DPC, BQ]
            dv = tokdec[st][:].rearrange("p (k d) -> p k d", d=DPC)
            for k in range(BPST):
                b = st * BPST + k
                nc.tensor.matmul(
                    out_ps[:],
                    dv[:, k, :],
                    sel_t[:, b * BQ : (b + 1) * BQ],
                    start=(first and k == 0),
                    stop=(last and k == BPST - 1),
                    skip_group_check=True,
                )

        emit_score(0)
        emit_score(1)
        emit_reduce(0)
        emit_score(2)
        emit_reduce(1)
        emit_score(3)
        emit_reduce(2)
        emit_sel(0, first=True)
        emit_sel(1)
        for k in range(NCLS):
            nc.tensor.matmul(
                out_ps[:],
                dclsT_t[:, k * DPC : (k + 1) * DPC],
                qclsT_t[:, k * BQ : (k + 1) * BQ],
                start=False,
                stop=False,
                skip_group_check=True,
            )
        emit_reduce(3)
        emit_sel(2)
        emit_sel(3, last=True)

        outsb = work.tile([DPC, BQ], F32, tag="outsb")
        nc.vector.tensor_copy(outsb[:], out_ps[:])
        nc.gpsimd.dma_start(out_d[:], outsb[:])

    nc.compile()
    return nc


_NC_CACHE = {}


def _get_nc(P):
    if P not in _NC_CACHE:
        _NC_CACHE[P] = build_nc(P)
    return _NC_CACHE[P]


def _build_layout(qid, did, qam):
    """Greedy vocab->bucket map balancing (a) active query tokens <= 128 per
    bucket and (b) the max per-(doc,bucket) position count (the pad P)."""
    sep = qam.sum(1) - 1
    w = qam.astype(np.float32).copy()
    w[np.arange(BQ), sep] = 0.0
    w[:, 0] = 0.0
    act = np.argwhere(w > 0)

    qcnt = np.zeros(VOCAB, np.int64)
    np.add.at(qcnt, qid[act[:, 0], act[:, 1]], 1)
    dcnt = np.zeros((VOCAB, BD), np.int64)
    for dd in range(BD):
        np.add.at(dcnt[:, dd], did[dd], 1)

    order = np.argsort(-(dcnt.max(1) * 1000 + qcnt * 100 + dcnt.sum(1)))
    present = (qcnt[order] > 0) | (dcnt[order].any(1))
    order = order[present]

    qload = np.zeros(NB, np.int64)
    dload = np.zeros((NB, BD), np.int64)
    g = np.zeros(VOCAB, np.int64)
    for v in order:
        cand = np.flatnonzero(qload + qcnt[v] <= 128)
        if len(cand) == 0:
            cand = np.arange(NB)
        nm = (dload[cand] + dcnt[v]).max(1)
        ss = ((dload[cand] + dcnt[v]) ** 2).sum(1)
        key = nm * (1 << 40) + ss * 256 + qload[cand]
        b = cand[np.argmin(key)]
        g[v] = b
        qload[b] += qcnt[v]
        dload[b] += dcnt[v]
    assert qload.max() <= 128

    # dense codes per bucket for distinct active query ids; sentinel after
    codetab = np.full((NB, VOCAB), -1, np.int64)
    nq = np.zeros(NB, np.int64)
    slots = [[] for _ in range(NB)]
    for q, i in act:
        v = int(qid[q, i])
        b = int(g[v])
        if codetab[b, v] < 0:
            codetab[b, v] = nq[b]
            nq[b] += 1
        slots[b].append((q, i))
    assert int(nq.max()) + 1 <= DIGB ** NDIG
    # doc-only ids -> per-bucket sentinel
    for b in range(NB):
        mask = codetab[b] < 0
        codetab[b, mask] = nq[b]

    P = int(dload.max()) + 1  # +1: every segment keeps a 512-seed pad col
    return g, codetab, P, slots, w


def _onehot_cols(codes, scale):
    """codes [N] int -> [KD, N] float32 one-hot of base-4 digits."""
    oh = np.zeros((KD, len(codes)), np.float32)
    idx = np.arange(len(codes))
    for t in range(NDIG):
        oh[t * DIGB + (codes // (DIGB ** t)) % DIGB, idx] = scale
    return oh


def make_in_maps(qte, dte, qce, dce, qid, did, qam):
    qid = np.asarray(qid).astype(np.int64)
    did = np.asarray(did).astype(np.int64)
    qam = np.asarray(qam).astype(np.int64)
    g, codetab, P, slots, w = _build_layout(qid, did, qam)
    NCOL = DPC * P

    # --- query side (shared across cores) ---
    lhsT = np.zeros((KK, NB, 128), NPDT)
    sel = np.zeros((128, NB, BQ), NPDT)
    lhsT[K, :, :] = NPDT(OFF)  # 512-seed row, all slots
    for b in range(NB):
        if not slots[b]:
            continue
        qq = np.array([s[0] for s in slots[b]])
        ii = np.array([s[1] for s in slots[b]])
        r = np.arange(len(qq))
        lhsT[0:TOK_D, b, r] = qte[qq, ii].T.astype(NPDT)
        codes = codetab[b, qid[qq, ii]]
        lhsT[TOK_D:K, b, r] = _onehot_cols(codes, C).astype(NPDT)
        sel[r, b, qq] = w[qq, ii].astype(NPDT)
    lhsT = lhsT.reshape(KK, NB * 128)
    sel = sel.reshape(128, NB * BQ)

    # CLS chunks + offset-correction chunk (cancels 512 * n_q)
    n_act = w.sum(1)  # [BQ] active tokens per query
    qclsT = np.zeros((128, NCLS, BQ), np.float32)
    qclsT[:, 0:6, :] = qce.T.reshape(6, 128, BQ).transpose(1, 0, 2)
    qclsT[:, 6, :] = n_act[None, :]
    qclsT = np.ascontiguousarray(qclsT.reshape(128, NCLS * BQ)).astype(NPDT)

    # --- doc side (per core) ---
    gb = g[did]                                   # [BD, LD] bucket per pos
    codes_pos = codetab[gb, did]                  # [BD, LD] code per pos
    dteT = dte.transpose(2, 0, 1).astype(NPDT)    # [32, BD, LD]
    oh_pos = np.zeros((KD, BD, LD), NPDT)
    for t in range(NDIG):
        dig = (codes_pos // (DIGB ** t)) % DIGB
        for dgt in range(DIGB):
            oh_pos[t * DIGB + dgt][dig == dgt] = 1.0

    in_maps = []
    for c in range(NCORES):
        docs = slice(c * DPC, (c + 1) * DPC)
        rhs = np.zeros((KK, NB, NCOL), NPDT)
        for dl in range(DPC):
            rhs[K, :, dl * P + P - 1] = 1.0  # 512-seed column per segment
            dd = c * DPC + dl
            cnt = np.zeros(NB, np.int64)
            order = np.argsort(gb[dd], kind="stable")
            for j in order:
                b = gb[dd, j]
                col = dl * P + cnt[b]
                cnt[b] += 1
                rhs[0:TOK_D, b, col] = dteT[:, dd, j]
                rhs[TOK_D:K, b, col] = oh_pos[:, dd, j]
        rhs = rhs.reshape(KK, NB * NCOL)

        dclsT = np.zeros((128, NCLS, DPC), np.float32)
        dclsT[:, 0:6, :] = dce[docs].T.reshape(6, 128, DPC).transpose(1, 0, 2)
        dclsT[:, 6, :] = -OFF / 128.0
        dclsT = np.ascontiguousarray(dclsT.reshape(128, NCLS * DPC)).astype(
            NPDT
        )

        in_maps.append(
            {
                "lhsT": np.ascontiguousarray(lhsT),
                "rhs": np.ascontiguousarray(rhs),
                "sel": np.ascontiguousarray(sel),
                "qclsT": qclsT,
                "dclsT": dclsT,
            }
        )
    return in_maps, P


def run(in_maps, P=None, trace=False, **kwargs):
    if P is None:
        P = in_maps[0]["rhs"].shape[1] // (NB * DPC)
    nc = _get_nc(P)
    return run_bass_kernel_spmd(
        nc, in_maps, core_ids=list(range(NCORES)), trace=trace, **kwargs
    )


def kernel(
    query_tok_embs,
    doc_tok_embs,
    query_cls_emb,
    doc_cls_emb,
    query_input_ids,
    doc_input_ids,
    query_attention_mask,
):
    qte = np.ascontiguousarray(np.asarray(query_tok_embs, np.float32))
    dte = np.ascontiguousarray(np.asarray(doc_tok_embs, np.float32))
    qce = np.ascontiguousarray(np.asarray(query_cls_emb, np.float32))
    dce = np.ascontiguousarray(np.asarray(doc_cls_emb, np.float32))
    qid = np.asarray(query_input_ids).astype(np.int64)
    did = np.asarray(doc_input_ids).astype(np.int64)
    qam = np.asarray(query_attention_mask).astype(np.int64)

    in_maps, P = make_in_maps(qte, dte, qce, dce, qid, did, qam)
    res = run(in_maps, P=P)
    out = np.concatenate([r["out"].T for r in res.results], axis=1)
    return np.ascontiguousarray(out.astype(np.float32))


# revision 20
# speedup vs baseline: 1.0976x; 1.0976x over previous
"""COIL-style sparse-attention scoring kernel for Trainium2 (8 NeuronCores).

Reference computation:
    scores[q,i,d,j] = <query_tok_embs[q,i], doc_tok_embs[d,j]>         (K=32)
    masked = where(query_ids[q,i]==doc_ids[d,j], scores, 0)
    tok    = masked.max(axis=j)                                        (192 -> 1)
    tok_scores[q,d] = sum_i w[q,i] * tok[q,i,d]    (w drops CLS + SEP)
    out = tok_scores + query_cls_emb @ doc_cls_emb.T

Strategy (bucketed): only same-token-id pairs survive the mask, so
partition the vocabulary into NB=16 buckets (host-side greedy balance) and
compute scores ONLY within a bucket.  Sharding is over docs (16 per core);
queries are replicated so each bucket's <=128 active query tokens fill one
full 128-row PE block.  Per (core, bucket) the doc side has ~12 positions
per doc, padded to P columns -> a [49, 128] x [49, 16*P] fp16 matmul.

  * Match masking via digit one-hots: within bucket b, distinct active
    query-token ids get dense codes (<= 129 incl. a doc-only sentinel),
    4 base-4 digits -> 16 one-hot dims (query side scaled by C=128, doc
    side 1.0).  The matmul output is aug = score + 128 * (#match digits);
    a full 4-digit match carries +512 while partials stay < 448 + |score|
    (|score| < 60 for this data).
  * Decode WITHOUT an activation: contraction row 48 is 512 on the query
    side and hits 1.0 exactly once per (doc,bucket) segment (in the pad
    column every segment has), so every segment contains an aug == 512
    sentinel and VectorE reduce_max yields  512 + relu(best matched score)
    directly in fp16.  The spurious 512*n_q offset (n_q = active tokens of
    query q) is cancelled by a 7th CLS chunk with dcls=-4, qcls=n_q.
  * Weighted token sum + CLS: fp16 matmuls accumulating into a TRANSPOSED
    [16, 64] PSUM tile (so the output DMA moves 16 fat rows, not 64 thin
    ones).  Host re-transposes and concatenates per-core outputs.

Per core: 16 score matmuls x 224 cols, 4 VectorE segmented reduces, 23
accumulation matmuls, one [16,64] copy + DMA out.  Scalar and Pool issue
DMAs only; the Act table load disappears from the preamble.
"""

import os
import numpy as np
import ml_dtypes
from contextlib import ExitStack

import concourse.bass as bass
import concourse.bacc as bacc
import concourse.mybir as mybir
import concourse.tile as tile
from concourse.bass_utils import run_bass_kernel_spmd

F32 = mybir.dt.float32
F16 = mybir.dt.float16
BF16 = mybir.dt.bfloat16

# problem shape (hardcoded per contract)
BQ, LQ, BD, LD, TOK_D, CLS_D = 64, 32, 128, 192, 32, 768
VOCAB = 5000
NCORES = 8
DPC = BD // NCORES          # 16 docs per core
NB = 16                     # vocab buckets == PE blocks
NDIG, DIGB = 4, 4           # 4 base-4 digits -> 16 one-hot dims, 256 codes
KD = NDIG * DIGB            # 16
K = TOK_D + KD              # 48 data contraction dims
KK = K + 1                  # +1 sentinel row (512-seed)
C = 128.0                   # per-digit match bonus
OFF = NDIG * C              # 512 full-match offset
NST = 4                     # supertiles (4 buckets each) = 1 PSUM tile each
BPST = NB // NST            # blocks per supertile
REG = 256                   # psum cols per block region (half-bank aligned)
NCLS = 7                    # 6 real CLS chunks + 1 offset-correction chunk

USE_BF16 = os.environ.get("K_DTYPE", "fp16") == "bf16"
DT = BF16 if USE_BF16 else F16
NPDT = ml_dtypes.bfloat16 if USE_BF16 else np.float16


def build_nc(P):
    """P = padded positions per (doc, bucket); NCOL = DPC*P streamed cols."""
    NCOL = DPC * P
    assert NCOL <= REG
    nc = bacc.Bacc(
        "TRN2",
        target_bir_lowering=False,
        debug=False,
        num_devices=NCORES,
    )

    lhsT_d = nc.dram_tensor("lhsT", [KK, NB * 128], DT, kind="ExternalInput")
    rhs_d = nc.dram_tensor("rhs", [KK, NB * NCOL], DT, kind="ExternalInput")
    sel_d = nc.dram_tensor("sel", [128, NB * BQ], DT, kind="ExternalInput")
    qclsT_d = nc.dram_tensor("qclsT", [128, NCLS * BQ], DT, kind="ExternalInput")
    dclsT_d = nc.dram_tensor("dclsT", [128, NCLS * DPC], DT, kind="ExternalInput")
    out_d = nc.dram_tensor("out", [DPC, BQ], F32, kind="ExternalOutput")

    with tile.TileContext(nc) as tc, ExitStack() as ctx:
        const = ctx.enter_context(tc.tile_pool(name="const", bufs=1))
        psum = ctx.enter_context(tc.tile_pool(name="psum", bufs=3, space="PSUM"))
        opsum = ctx.enter_context(tc.tile_pool(name="opsum", bufs=1, space="PSUM"))
        work = ctx.enter_context(tc.tile_pool(name="work", bufs=1))

        lhsT_t = const.tile([KK, NB * 128], DT, tag="lhsT")
        rhs_t = const.tile([KK, NB * NCOL], DT, tag="rhs")
        sel_t = const.tile([128, NB * BQ], DT, tag="sel")
        qclsT_t = const.tile([128, NCLS * BQ], DT, tag="qclsT")
        dclsT_t = const.tile([128, NCLS * DPC], DT, tag="dclsT")

        # --- input DMAs.  All queues share the same 16 hw DMA engines
        # (~145 B/ns aggregate), so order strictly by consumption: rhs +
        # lhsT first on two queues, then sel, then CLS.  gpsimd keeps only
        # the output (so it is idle + uncontended at the end).
        nc.scalar.dma_start(rhs_t[:, 0:NCOL], rhs_d[:, 0:NCOL])
        nc.sync.dma_start(lhsT_t[:, 0 : 4 * 128], lhsT_d[:, 0 : 4 * 128])
        nc.scalar.dma_start(rhs_t[:, NCOL : 5 * NCOL], rhs_d[:, NCOL : 5 * NCOL])
        nc.sync.dma_start(lhsT_t[:, 4 * 128 : 10 * 128], lhsT_d[:, 4 * 128 : 10 * 128])
        nc.scalar.dma_start(
            rhs_t[:, 5 * NCOL : 10 * NCOL], rhs_d[:, 5 * NCOL : 10 * NCOL]
        )
        nc.sync.dma_start(lhsT_t[:, 10 * 128 :], lhsT_d[:, 10 * 128 :])
        nc.scalar.dma_start(rhs_t[:, 10 * NCOL :], rhs_d[:, 10 * NCOL :])
        nc.sync.dma_start(sel_t[:, 0 : 8 * BQ], sel_d[:, 0 : 8 * BQ])
        nc.scalar.dma_start(sel_t[:, 8 * BQ :], sel_d[:, 8 * BQ :])
        nc.sync.dma_start(qclsT_t[:], qclsT_d[:])
        nc.scalar.dma_start(dclsT_t[:], dclsT_d[:])

        # --- emission: score supertiles pipelined with the DVE reduces and
        # the accumulation matmuls, ordered so the PE never head-of-line
        # blocks and the tail after the last score matmul is short.
        ps_tiles = [None] * NST
        tokdec = [None] * NST
        out_ps = opsum.tile([DPC, BQ], F32, tag="out_ps")

        def emit_score(st):
            ps = psum.tile([128, BPST, REG], F32, tag="score")
            ps_tiles[st] = ps
            for k in range(BPST):
                b = st * BPST + k
                nc.tensor.matmul(
                    ps[:, k, 0:NCOL],
                    lhsT_t[:, b * 128 : (b + 1) * 128],
                    rhs_t[:, b * NCOL : (b + 1) * NCOL],
                    start=True,
                    stop=True,
                )

        def emit_reduce(st):
            ps = ps_tiles[st]
            v = ps[:, :, 0:NCOL].rearrange("p k (d j) -> p k d j", j=P)
            dec = work.tile([128, BPST * DPC], DT, tag=f"dec{st}")
            nc.vector.reduce_max(
                dec[:].rearrange("p (k d) -> p k d", d=DPC),
                v,
                axis=mybir.AxisListType.X,
            )
            tokdec[st] = dec

        def emit_sel(st, first=False, last=False):
            # transposed accumulation: tokdec stationary, sel moving ->
            # out [DPC, BQ]
            dv = tokdec[st][:].rearrange("p (k d) -> p k d", d=DPC)
            for k in range(BPST):
                b = st * BPST + k
                nc.tensor.matmul(
                    out_ps[:],
                    dv[:, k, :],
                    sel_t[:, b * BQ : (b + 1) * BQ],
                    start=(first and k == 0),
                    stop=(last and k == BPST - 1),
                    skip_group_check=True,
                )

        emit_score(0)
        emit_score(1)
        emit_reduce(0)
        emit_score(2)
        emit_reduce(1)
        emit_score(3)
        emit_reduce(2)
        emit_sel(0, first=True)
        emit_sel(1)
        for k in range(NCLS):
            nc.tensor.matmul(
                out_ps[:],
                dclsT_t[:, k * DPC : (k + 1) * DPC],
                qclsT_t[:, k * BQ : (k + 1) * BQ],
                start=False,
                stop=False,
                skip_group_check=True,
            )
        emit_reduce(3)
        emit_sel(2)
        emit_sel(3, last=True)

        outsb = work.tile([DPC, BQ], F32, tag="outsb")
        nc.vector.tensor_copy(outsb[:], out_ps[:])
        nc.gpsimd.dma_start(out_d[:], outsb[:])

    nc.compile()
    return nc


_NC_CACHE = {}


def _get_nc(P):
    if P not in _NC_CACHE:
        _NC_CACHE[P] = build_nc(P)
    return _NC_CACHE[P]


def _build_layout(qid, did, qam):
    """Greedy vocab->bucket map balancing (a) active query tokens <= 128 per
    bucket and (b) the max per-(doc,bucket) position count (the pad P)."""
    sep = qam.sum(1) - 1
    w = qam.astype(np.float32).copy()
    w[np.arange(BQ), sep] = 0.0
    w[:, 0] = 0.0
    act = np.argwhere(w > 0)

    qcnt = np.zeros(VOCAB, np.int64)
    np.add.at(qcnt, qid[act[:, 0], act[:, 1]], 1)
    dcnt = np.zeros((VOCAB, BD), np.int64)
    for dd in range(BD):
        np.add.at(dcnt[:, dd], did[dd], 1)

    order = np.argsort(-(dcnt.max(1) * 1000 + qcnt * 100 + dcnt.sum(1)))
    present = (qcnt[order] > 0) | (dcnt[order].any(1))
    order = order[present]

    qload = np.zeros(NB, np.int64)
    dload = np.zeros((NB, BD), np.int64)
    g = np.zeros(VOCAB, np.int64)
    for v in order:
        cand = np.flatnonzero(qload + qcnt[v] <= 128)
        if len(cand) == 0:
            cand = np.arange(NB)
        nm = (dload[cand] + dcnt[v]).max(1)
        ss = ((dload[cand] + dcnt[v]) ** 2).sum(1)
        key = nm * (1 << 40) + ss * 256 + qload[cand]
        b = cand[np.argmin(key)]
        g[v] = b
        qload[b] += qcnt[v]
        dload[b] += dcnt[v]
    assert qload.max() <= 128

    # dense codes per bucket for distinct active query ids; sentinel after
    codetab = np.full((NB, VOCAB), -1, np.int64)
    nq = np.zeros(NB, np.int64)
    slots = [[] for _ in range(NB)]
    for q, i in act:
        v = int(qid[q, i])
        b = int(g[v])
        if codetab[b, v] < 0:
            codetab[b, v] = nq[b]
            nq[b] += 1
        slots[b].append((q, i))
    assert int(nq.max()) + 1 <= DIGB ** NDIG
    # doc-only ids -> per-bucket sentinel
    for b in range(NB):
        mask = codetab[b] < 0
        codetab[b, mask] = nq[b]

    P = int(dload.max()) + 1  # +1: every segment keeps a 512-seed pad col
    return g, codetab, P, slots, w


def _onehot_cols(codes, scale):
    """codes [N] int -> [KD, N] float32 one-hot of base-4 digits."""
    oh = np.zeros((KD, len(codes)), np.float32)
    idx = np.arange(len(codes))
    for t in range(NDIG):
        oh[t * DIGB + (codes // (DIGB ** t)) % DIGB, idx] = scale
    return oh


def make_in_maps(qte, dte, qce, dce, qid, did, qam):
    qid = np.asarray(qid).astype(np.int64)
    did = np.asarray(did).astype(np.int64)
    qam = np.asarray(qam).astype(np.int64)
    g, codetab, P, slots, w = _build_layout(qid, did, qam)
    NCOL = DPC * P

    # --- query side (shared across cores) ---
    lhsT = np.zeros((KK, NB, 128), NPDT)
    sel = np.zeros((128, NB, BQ), NPDT)
    lhsT[K, :, :] = NPDT(OFF)  # 512-seed row, all slots
    for b in range(NB):
        if not slots[b]:
            continue
        qq = np.array([s[0] for s in slots[b]])
        ii = np.array([s[1] for s in slots[b]])
        r = np.arange(len(qq))
        lhsT[0:TOK_D, b, r] = qte[qq, ii].T.astype(NPDT)
        codes = codetab[b, qid[qq, ii]]
        lhsT[TOK_D:K, b, r] = _onehot_cols(codes, C).astype(NPDT)
        sel[r, b, qq] = w[qq, ii].astype(NPDT)
    lhsT = lhsT.reshape(KK, NB * 128)
    sel = sel.reshape(128, NB * BQ)

    # CLS chunks + offset-correction chunk (cancels 512 * n_q)
    n_act = w.sum(1)  # [BQ] active tokens per query
    qclsT = np.zeros((128, NCLS, BQ), np.float32)
    qclsT[:, 0:6, :] = qce.T.reshape(6, 128, BQ).transpose(1, 0, 2)
    qclsT[:, 6, :] = n_act[None, :]
    qclsT = np.ascontiguousarray(qclsT.reshape(128, NCLS * BQ)).astype(NPDT)

    # --- doc side (per core) ---
    gb = g[did]                                   # [BD, LD] bucket per pos
    codes_pos = codetab[gb, did]                  # [BD, LD] code per pos
    dteT = dte.transpose(2, 0, 1).astype(NPDT)    # [32, BD, LD]
    oh_pos = np.zeros((KD, BD, LD), NPDT)
    for t in range(NDIG):
        dig = (codes_pos // (DIGB ** t)) % DIGB
        for dgt in range(DIGB):
            oh_pos[t * DIGB + dgt][dig == dgt] = 1.0

    in_maps = []
    for c in range(NCORES):
        docs = slice(c * DPC, (c + 1) * DPC)
        rhs = np.zeros((KK, NB, NCOL), NPDT)
        for dl in range(DPC):
            rhs[K, :, dl * P + P - 1] = 1.0  # 512-seed column per segment
            dd = c * DPC + dl
            cnt = np.zeros(NB, np.int64)
            order = np.argsort(gb[dd], kind="stable")
            for j in order:
                b = gb[dd, j]
                col = dl * P + cnt[b]
                cnt[b] += 1
                rhs[0:TOK_D, b, col] = dteT[:, dd, j]
                rhs[TOK_D:K, b, col] = oh_pos[:, dd, j]
        rhs = rhs.reshape(KK, NB * NCOL)

        dclsT = np.zeros((128, NCLS, DPC), np.float32)
        dclsT[:, 0:6, :] = dce[docs].T.reshape(6, 128, DPC).transpose(1, 0, 2)
        dclsT[:, 6, :] = -OFF / 128.0
        dclsT = np.ascontiguousarray(dclsT.reshape(128, NCLS * DPC)).astype(
            NPDT
        )

        in_maps.append(
            {
                "lhsT": np.ascontiguousarray(lhsT),
                "rhs": np.ascontiguousarray(rhs),
                "sel": np.ascontiguousarray(sel),
                "qclsT": qclsT,
                "dclsT": dclsT,
            }
        )
    return in_maps, P


def run(in_maps, P=None, trace=False, **kwargs):
    if P is None:
        P = in_maps[0]["rhs"].shape[1] // (NB * DPC)
    nc = _get_nc(P)
    return run_bass_kernel_spmd(
        nc, in_maps, core_ids=list(range(NCORES)), trace=trace, **kwargs
    )


def kernel(
    query_tok_embs,
    doc_tok_embs,
    query_cls_emb,
    doc_cls_emb,
    query_input_ids,
    doc_input_ids,
    query_attention_mask,
):
    qte = np.ascontiguousarray(np.asarray(query_tok_embs, np.float32))
    dte = np.ascontiguousarray(np.asarray(doc_tok_embs, np.float32))
    qce = np.ascontiguousarray(np.asarray(query_cls_emb, np.float32))
    dce = np.ascontiguousarray(np.asarray(doc_cls_emb, np.float32))
    qid = np.asarray(query_input_ids).astype(np.int64)
    did = np.asarray(doc_input_ids).astype(np.int64)
    qam = np.asarray(query_attention_mask).astype(np.int64)

    in_maps, P = make_in_maps(qte, dte, qce, dce, qid, did, qam)
    res = run(in_maps, P=P)
    out = np.concatenate([r["out"].T for r in res.results], axis=1)
    return np.ascontiguousarray(out.astype(np.float32))
